# revision 1
# baseline (speedup 1.0000x reference)
"""MixHop GNN (nn_MixHopNetwork_75299366633514) on 8 TRN2 NeuronCores.

Strategy (self-contained; shapes hardcoded for the full problem):
  - Nodes sharded contiguously: core c owns rows [c*12500, (c+1)*12500),
    padded to 12800 local rows (100 blocks of 128).
  - Restructured math:
      r_i = relu(X @ Wu_i + bu_i)                       (dense, bf16)
      hop1: [A r1 | A r2]     = A @ [r1 | r2]           (512-wide spmm)
      hop2: [A^2 r2]          = A @ (A r2)              (256-wide spmm)
      abs1 = [r0 | A r1 | A^2 r2]                       (768 cols, 256-padded)
      B_i  = abs1 @ Wb_i                                (dense)
      hop3: [A B1 | A B2], hop4: [A^2 B2]
      abs2 = [B0+bb0 | A B1+bb1 | A^2 B2+bb2]
      out  = log_softmax(abs2 @ fc_w + fc_b)
  - spmm per core: edges sharded by destination; per destination-block (128
    rows) and source-window (int16 index range), dma_gather pulls source rows
    from AllGathered operand buffers (HBM), a one-hot(val) matrix built by two
    DVE ops turns each 128-edge chunk into a PE matmul accumulating into PSUM.
  - AllGathers are split into 4 row-quarters for compute/comm overlap.
"""

import math
import numpy as np
import ml_dtypes

bf16 = ml_dtypes.bfloat16


class Cfg:
    def __init__(self, N, F, H, HP, CL, rpc_raw, rpc, qrows, deg_scale=None):
        self.NC = 8
        self.N = N            # real nodes
        self.F = F            # input features (multiple of 128)
        self.H = H            # real hidden per power
        self.HP = HP          # padded hidden (multiple of 128, >=H)
        self.W2 = 2 * HP      # fused double width
        self.CL = CL          # classes
        self.RPC_RAW = rpc_raw  # real rows per core
        self.RPC = rpc        # padded rows per core (multiple of 512)
        self.QROWS = qrows    # rows per quarter (RPC/4, multiple of 128)
        self.NBLK = rpc // 128
        self.NWIN = 4
        self.WIN_ROWS = self.NC * qrows  # rows per AG window buffer
        assert rpc == 4 * qrows and qrows % 128 == 0
        assert self.WIN_ROWS <= 32768


FULL = Cfg(N=100000, F=512, H=200, HP=256, CL=40,
           rpc_raw=12500, rpc=12800, qrows=3200)


# ---------------------------------------------------------------- host side

def preprocess(cfg, features, adj_row, adj_col, adj_val, Wu, bu, Wb, bb, fc_w, fc_b):
    """Pure-numpy preprocessing -> per-core in_maps + shared segment table."""
    NC, RPCR, RPC, QROWS = cfg.NC, cfg.RPC_RAW, cfg.RPC, cfg.QROWS
    NBLK, NWIN = cfg.NBLK, cfg.NWIN
    H, HP, F, CL = cfg.H, cfg.HP, cfg.F, cfg.CL

    row = adj_row.astype(np.int64)
    col = adj_col.astype(np.int64)
    val = adj_val.astype(np.float32)

    core = row // RPCR
    dl = row - core * RPCR           # local dest row 0..RPCR-1
    b = dl >> 7                      # dest block
    doff = dl & 127
    cs = col // RPCR
    ls = col - cs * RPCR
    w = ls // QROWS                  # source window
    idx = cs * QROWS + (ls - w * QROWS)  # row within window buffer

    # counts per (core, b, w)
    key = (core * NBLK + b) * NWIN + w
    counts = np.bincount(key, minlength=NC * NBLK * NWIN).reshape(NC, NBLK, NWIN)
    Cs = np.maximum(1, -(-counts.max(axis=0) // 128))  # [NBLK, NWIN] chunks, >=1
    seg_off = np.zeros((NBLK, NWIN), np.int64)         # chunk offsets
    flat = Cs.reshape(-1)
    seg_off.reshape(-1)[1:] = np.cumsum(flat)[:-1]
    TOT = int(flat.sum()) * 128                        # padded slots

    # slot of each edge: seg_off[b,w]*128 + rank within its (core,b,w) group
    order = np.lexsort((w, b, core))
    so_r, so_b, so_w = core[order], b[order], w[order]
    gkey = (so_r * NBLK + so_b) * NWIN + so_w
    grp_start = np.zeros(len(gkey), np.int64)
    new_grp = np.ones(len(gkey), bool)
    new_grp[1:] = gkey[1:] != gkey[:-1]
    starts_idx = np.nonzero(new_grp)[0]
    grp_start[starts_idx] = starts_idx
    grp_start = np.maximum.accumulate(grp_start)
    cum = np.arange(len(gkey)) - grp_start
    slot = seg_off[so_b, so_w] * 128 + cum

    # build padded per-core meta arrays
    idx_pad = np.zeros((NC, TOT), np.int16)
    doff_pad = np.zeros((NC, TOT), np.int16)
    val_pad = np.zeros((NC, TOT), np.float32)
    idx_pad[so_r, slot] = idx[order].astype(np.int16)
    doff_pad[so_r, slot] = doff[order].astype(np.int16)
    val_pad[so_r, slot] = val[order]

    # device layouts
    idx_dev = np.zeros((NC, 128, TOT // 16), np.int16)
    blk = np.transpose(idx_pad.reshape(NC, TOT // 16, 16), (0, 2, 1))  # [NC,16,TOT/16]
    for k in range(8):
        idx_dev[:, 16 * k:16 * (k + 1), :] = blk
    doff_dev = np.ascontiguousarray(
        np.transpose(doff_pad.reshape(NC, TOT // 128, 128), (0, 2, 1))
    ).astype(bf16)
    val_dev = np.ascontiguousarray(
        np.transpose(val_pad.reshape(NC, TOT // 128, 128), (0, 2, 1))
    ).astype(bf16)

    # features: per-core [F, RPC] bf16 (transposed, padded)
    featsT = np.zeros((NC, F, RPC), bf16)
    for c in range(NC):
        featsT[c, :, :RPCR] = features[c * RPCR:(c + 1) * RPCR].T.astype(bf16)

    # weights, padded
    Wu_p = np.zeros((3, F, HP), bf16)
    Wu_p[:, :, :H] = Wu.astype(bf16)
    bu_rep = np.zeros((3, 128, HP), np.float32)
    bu_rep[:, :, :H] = bu[:, None, :]
    # Wb rows: [0:H]->0:H, [H:2H]->HP:HP+H, [2H:3H]->2HP:2HP+H
    Wb_p = np.zeros((3, 3 * HP, HP), bf16)
    fcw_p = np.zeros((3 * HP, CL), bf16)
    for j in range(3):
        Wb_p[:, j * HP:j * HP + H, :H] = Wb[:, j * H:(j + 1) * H, :].astype(bf16)
        fcw_p[j * HP:j * HP + H, :] = fc_w[j * H:(j + 1) * H, :].astype(bf16)
    bb_rep = np.zeros((3, 128, HP), np.float32)
    bb_rep[:, :, :H] = bb[:, None, :]
    fcb_rep = np.broadcast_to(fc_b.astype(np.float32), (128, CL)).copy()
    iota = np.broadcast_to(np.arange(128, dtype=np.float32), (128, 128)).astype(bf16)

    in_maps = []
    for c in range(NC):
        in_maps.append({
            "featsT": featsT[c],
            "meta_idx": idx_dev[c],
            "meta_doff": doff_dev[c],
            "meta_val": val_dev[c],
            "Wu_p": Wu_p, "bu_rep": bu_rep.astype(np.float32),
            "Wb_p": Wb_p, "bb_rep": bb_rep.astype(np.float32),
            "fcw_p": fcw_p, "fcb_rep": fcb_rep,
            "iota": np.asarray(iota),
        })
    segs = [[(int(seg_off[bb_, ww]), int(Cs[bb_, ww])) for ww in range(NWIN)]
            for bb_ in range(NBLK)]
    return in_maps, segs, TOT


# -------------------------------------------------------------- device side

def build_nc(cfg, segs, TOT, MAXC=5):
    import concourse.bacc as bacc
    import concourse.mybir as mybir
    import concourse.tile as tile

    dt = mybir.dt
    NC, RPC, QROWS = cfg.NC, cfg.RPC, cfg.QROWS
    NBLK, NWIN, WIN_ROWS = cfg.NBLK, cfg.NWIN, cfg.WIN_ROWS
    F, HP, W2, CL = cfg.F, cfg.HP, cfg.W2, cfg.CL
    KF = F // 128          # feature k-chunks
    KA = 3 * HP // 128     # abstract k-chunks
    MG = 10 if NBLK % 10 == 0 else 2   # m-tiles per group
    NG = NBLK // MG
    RG = [list(range(g * MG, (g + 1) * MG)) for g in range(NG)]
    ALL = list(range(NC))

    nc = bacc.Bacc("TRN2", target_bir_lowering=False, debug=False, num_devices=NC)

    featsT = nc.dram_tensor("featsT", [F, RPC], dt.bfloat16, kind="ExternalInput")
    meta_idx = nc.dram_tensor("meta_idx", [128, TOT // 16], dt.int16, kind="ExternalInput")
    meta_doff = nc.dram_tensor("meta_doff", [128, TOT // 128], dt.bfloat16, kind="ExternalInput")
    meta_val = nc.dram_tensor("meta_val", [128, TOT // 128], dt.bfloat16, kind="ExternalInput")
    Wu_p = nc.dram_tensor("Wu_p", [3, F, HP], dt.bfloat16, kind="ExternalInput")
    bu_rep = nc.dram_tensor("bu_rep", [3, 128, HP], dt.float32, kind="ExternalInput")
    Wb_p = nc.dram_tensor("Wb_p", [3, 3 * HP, HP], dt.bfloat16, kind="ExternalInput")
    bb_rep = nc.dram_tensor("bb_rep", [3, 128, HP], dt.float32, kind="ExternalInput")
    fcw_p = nc.dram_tensor("fcw_p", [3 * HP, CL], dt.bfloat16, kind="ExternalInput")
    fcb_rep = nc.dram_tensor("fcb_rep", [128, CL], dt.float32, kind="ExternalInput")
    iota_in = nc.dram_tensor("iota", [128, 128], dt.bfloat16, kind="ExternalInput")
    y_out = nc.dram_tensor("y_out", [RPC, CL], dt.float32, kind="ExternalOutput")

    Cmax = max(sum(C for (_, C) in bw) for bw in segs)

    with tile.TileContext(nc) as tc:
        with (
            tc.tile_pool(name="const", bufs=1) as cpool,
            tc.tile_pool(name="dram", bufs=1, space="DRAM") as dram,
        ):
            # ---- resident constants
            iota_t = cpool.tile([128, 128], dt.bfloat16)
            nc.sync.dma_start(iota_t[:], iota_in[:])
            idx_t = cpool.tile([128, TOT // 16], dt.int16)
            nc.sync.dma_start(idx_t[:], meta_idx[:])
            doff_t = cpool.tile([128, TOT // 128], dt.bfloat16)
            nc.sync.dma_start(doff_t[:], meta_doff[:])
            val_t = cpool.tile([128, TOT // 128], dt.bfloat16)
            nc.sync.dma_start(val_t[:], meta_val[:])
            wu_t = cpool.tile([128, 3, KF, HP], dt.bfloat16)
            for i in range(3):
                for kc in range(KF):
                    nc.sync.dma_start(wu_t[:, i, kc, :], Wu_p[i, kc * 128:(kc + 1) * 128, :])
            wb_t = cpool.tile([128, 3, KA, HP], dt.bfloat16)
            for i in range(3):
                for kc in range(KA):
                    nc.sync.dma_start(wb_t[:, i, kc, :], Wb_p[i, kc * 128:(kc + 1) * 128, :])
            fcw_t = cpool.tile([128, KA, CL], dt.bfloat16)
            for kc in range(KA):
                nc.sync.dma_start(fcw_t[:, kc, :], fcw_p[kc * 128:(kc + 1) * 128, :])
            bu_t = cpool.tile([128, 3, HP], dt.float32)
            bb_t = cpool.tile([128, 3, HP], dt.float32)
            for i in range(3):
                nc.sync.dma_start(bu_t[:, i, :], bu_rep[i])
                nc.sync.dma_start(bb_t[:, i, :], bb_rep[i])
            fcb_t = cpool.tile([128, CL], dt.float32)
            nc.sync.dma_start(fcb_t[:], fcb_rep[:])

            # ---- DRAM intermediates
            abs1 = dram.tile([RPC, 3 * HP], dt.bfloat16)
            abs2 = dram.tile([RPC, 3 * HP], dt.bfloat16)
            agin1 = [dram.tile([QROWS, W2], dt.bfloat16, name=f"agin1_{q}") for q in range(4)]
            agbuf1 = [dram.tile([WIN_ROWS, W2], dt.bfloat16, addr_space="Shared", name=f"agbuf1_{q}") for q in range(4)]
            agin2 = [dram.tile([QROWS, HP], dt.bfloat16, name=f"agin2_{q}") for q in range(4)]
            agbuf2 = [dram.tile([WIN_ROWS, HP], dt.bfloat16, addr_space="Shared", name=f"agbuf2_{q}") for q in range(4)]
            agin3 = [dram.tile([QROWS, W2], dt.bfloat16, name=f"agin3_{q}") for q in range(4)]
            agbuf3 = [dram.tile([WIN_ROWS, W2], dt.bfloat16, addr_space="Shared", name=f"agbuf3_{q}") for q in range(4)]
            agin4 = [dram.tile([QROWS, HP], dt.bfloat16, name=f"agin4_{q}") for q in range(4)]
            agbuf4 = [dram.tile([WIN_ROWS, HP], dt.bfloat16, addr_space="Shared", name=f"agbuf4_{q}") for q in range(4)]

            def allgather(agin_q, agbuf_q):
                for q in range(4):
                    nc.gpsimd.collective_compute(
                        "AllGather", mybir.AluOpType.bypass,
                        replica_groups=[ALL],
                        ins=[agin_q[q][:].opt()],
                        outs=[agbuf_q[q][:].opt()],
                    )

            # ================= D1: r_i = relu(X @ Wu_i + bu_i)
            with (
                tc.tile_pool(name="d1", bufs=2) as dp,
                tc.tile_pool(name="d1ps", bufs=2, space="PSUM") as pp,
            ):
                for g in range(NG):
                    xt = dp.tile([128, KF, MG * 128], dt.bfloat16, name="xt", tag="xt", bufs=2)
                    for kc in range(KF):
                        nc.sync.dma_start(
                            xt[:, kc, :],
                            featsT[kc * 128:(kc + 1) * 128, g * MG * 128:(g + 1) * MG * 128])
                    for ml in range(MG):
                        m = g * MG + ml
                        ps = pp.tile([128, 3, HP], dt.float32, name="d1p", tag="d1p", bufs=2)
                        for i in range(3):
                            for kc in range(KF):
                                nc.tensor.matmul(
                                    ps[:, i, :],
                                    xt[:, kc, ml * 128:(ml + 1) * 128],
                                    wu_t[:, i, kc, :],
                                    start=(kc == 0), stop=(kc == KF - 1))
                        rt = dp.tile([128, 3, HP], dt.bfloat16, name="rt", tag="rt", bufs=3)
                        for i in range(3):
                            nc.vector.tensor_add(rt[:, i, :], ps[:, i, :], bu_t[:, i, :])
                            nc.vector.tensor_relu(rt[:, i, :], rt[:, i, :])
                        q, lr = divmod(m * 128, QROWS)
                        nc.sync.dma_start(abs1[m * 128:(m + 1) * 128, 0:HP], rt[:, 0, :])
                        nc.sync.dma_start(agin1[q][lr:lr + 128, 0:HP], rt[:, 1, :])
                        nc.sync.dma_start(agin1[q][lr:lr + 128, HP:W2], rt[:, 2, :])
            allgather(agin1, agbuf1)

            # ================= hop phases
            def hop(agbuf_q, width, emits):
                """emits: list of (col0, dst_kind, dst, dcol, bias_i or None)
                   dst_kind: 'abs' -> dst[rows, dcol:dcol+HP]
                             'ag'  -> dst[q][lr:lr+128, dcol:dcol+HP]"""
                with (
                    tc.tile_pool(name="hp", bufs=2) as hp,
                    tc.tile_pool(name="hps", bufs=4, space="PSUM") as hpp,
                ):
                    for b in range(NBLK):
                        bw = segs[b]
                        Cb = sum(C for (_, C) in bw)
                        gt = hp.tile([128, Cb, width], dt.bfloat16, name="gt", tag="gt",
                                     bufs=2)
                        co = 0
                        for w in range(NWIN):
                            off, C = bw[w]
                            while C > 0:
                                Cg = min(C, MAXC)
                                nc.gpsimd.dma_gather(
                                    gt[:, co:co + Cg, :],
                                    agbuf_q[w][:],
                                    idx_t[:, off * 8:(off + Cg) * 8],
                                    num_idxs=Cg * 128,
                                    num_idxs_reg=Cg * 128,
                                    elem_size=width)
                                co += Cg
                                off += Cg
                                C -= Cg
                        oh = hp.tile([128, Cb, 128], dt.bfloat16, name="oh", tag="oh",
                                     bufs=2)
                        c0 = bw[0][0]
                        nc.vector.tensor_tensor(
                            oh[:],
                            iota_t[:].unsqueeze(1).broadcast_to([128, Cb, 128]),
                            doff_t[:, c0:c0 + Cb].unsqueeze(2).broadcast_to([128, Cb, 128]),
                            mybir.AluOpType.is_equal)
                        nc.vector.tensor_tensor(
                            oh[:], oh[:],
                            val_t[:, c0:c0 + Cb].unsqueeze(2).broadcast_to([128, Cb, 128]),
                            mybir.AluOpType.mult)
                        ps = hpp.tile([128, width], dt.float32, name="hpsum", tag="hpsum",
                                      bufs=4)
                        for c in range(Cb):
                            nc.tensor.matmul(ps[:], oh[:, c, :], gt[:, c, :],
                                             start=(c == 0), stop=(c == Cb - 1))
                        q, lr = divmod(b * 128, QROWS)
                        for (col0, kind, dst, dcol, bias_i) in emits:
                            ot = hp.tile([128, HP], dt.bfloat16, name="ot", tag="ot", bufs=4)
                            if bias_i is None:
                                nc.scalar.activation(ot[:], ps[:, col0:col0 + HP],
                                                     mybir.ActivationFunctionType.Copy)
                            else:
                                nc.vector.tensor_add(ot[:], ps[:, col0:col0 + HP],
                                                     bb_t[:, bias_i, :])
                            if kind == 'abs':
                                nc.sync.dma_start(dst[b * 128:(b + 1) * 128, dcol:dcol + HP], ot[:])
                            else:
                                nc.sync.dma_start(dst[q][lr:lr + 128, dcol:dcol + HP], ot[:])

            # H1: A[r1|r2] -> Ar1 to abs1 cols HP:2HP ; Ar2 to agin2
            hop(agbuf1, W2, [(0, 'abs', abs1, HP, None), (HP, 'ag', agin2, 0, None)])
            allgather(agin2, agbuf2)
            # H2: A^2 r2 -> abs1 cols 2HP:3HP
            hop(agbuf2, HP, [(0, 'abs', abs1, 2 * HP, None)])

            # ================= D2: B_i = abs1 @ Wb_i
            with (
                tc.tile_pool(name="d2", bufs=2) as dp,
                tc.tile_pool(name="d2ps", bufs=2, space="PSUM") as pp,
            ):
                for g in range(NG):
                    a1t = dp.tile([128, KA, MG * 128], dt.bfloat16, name="a1t", tag="a1t", bufs=2)
                    for kc in range(KA):
                        nc.sync.dma_start(
                            a1t[:, kc, :],
                            abs1[g * MG * 128:(g + 1) * MG * 128, kc * 128:(kc + 1) * 128],
                            transpose=True)
                    for ml in range(MG):
                        m = g * MG + ml
                        ps = pp.tile([128, 3, HP], dt.float32, name="d2p", tag="d2p", bufs=2)
                        for i in range(3):
                            for kc in range(KA):
                                nc.tensor.matmul(
                                    ps[:, i, :],
                                    a1t[:, kc, ml * 128:(ml + 1) * 128],
                                    wb_t[:, i, kc, :],
                                    start=(kc == 0), stop=(kc == KA - 1))
                        bt = dp.tile([128, 3, HP], dt.bfloat16, name="bt", tag="bt", bufs=3)
                        nc.vector.tensor_add(bt[:, 0, :], ps[:, 0, :], bb_t[:, 0, :])
                        nc.scalar.activation(bt[:, 1, :], ps[:, 1, :],
                                             mybir.ActivationFunctionType.Copy)
                        nc.scalar.activation(bt[:, 2, :], ps[:, 2, :],
                                             mybir.ActivationFunctionType.Copy)
                        q, lr = divmod(m * 128, QROWS)
                        nc.sync.dma_start(abs2[m * 128:(m + 1) * 128, 0:HP], bt[:, 0, :])
                        nc.sync.dma_start(agin3[q][lr:lr + 128, 0:HP], bt[:, 1, :])
                        nc.sync.dma_start(agin3[q][lr:lr + 128, HP:W2], bt[:, 2, :])
            allgather(agin3, agbuf3)

            # H3: A[B1|B2] -> AB1+bb1 to abs2 ; AB2 to agin4
            hop(agbuf3, W2, [(0, 'abs', abs2, HP, 1), (HP, 'ag', agin4, 0, None)])
            allgather(agin4, agbuf4)
            # H4: A^2 B2 + bb2 -> abs2 cols 2HP:3HP
            hop(agbuf4, HP, [(0, 'abs', abs2, 2 * HP, 2)])

            # ================= D3: logits + log_softmax
            with (
                tc.tile_pool(name="d3", bufs=2) as dp,
                tc.tile_pool(name="d3ps", bufs=2, space="PSUM") as pp,
            ):
                for g in range(NG):
                    a2t = dp.tile([128, KA, MG * 128], dt.bfloat16, name="a2t", tag="a2t", bufs=2)
                    for kc in range(KA):
                        nc.sync.dma_start(
                            a2t[:, kc, :],
                            abs2[g * MG * 128:(g + 1) * MG * 128, kc * 128:(kc + 1) * 128],
                            transpose=True)
                    for ml in range(MG):
                        m = g * MG + ml
                        ps = pp.tile([128, CL], dt.float32, name="d3p", tag="d3p", bufs=2)
                        for kc in range(KA):
                            nc.tensor.matmul(
                                ps[:],
                                a2t[:, kc, ml * 128:(ml + 1) * 128],
                                fcw_t[:, kc, :],
                                start=(kc == 0), stop=(kc == KA - 1))
                        lg = dp.tile([128, CL], dt.float32, name="lg", tag="lg", bufs=3)
                        nc.vector.tensor_add(lg[:], ps[:], fcb_t[:])
                        mx = dp.tile([128, 1], dt.float32, name="mx", tag="mx", bufs=3)
                        nc.vector.tensor_reduce(mx[:], lg[:], mybir.AxisListType.X,
                                                mybir.AluOpType.max, negate=True)
                        ex = dp.tile([128, CL], dt.float32, name="ex", tag="ex", bufs=3)
                        sm = dp.tile([128, 1], dt.float32, name="sm", tag="sm", bufs=3)
                        nc.scalar.activation(ex[:], lg[:], mybir.ActivationFunctionType.Exp,
                                             bias=mx[:], accum_out=sm[:])
                        ln = dp.tile([128, 1], dt.float32, name="ln", tag="ln", bufs=3)
                        nc.scalar.activation(ln[:], sm[:], mybir.ActivationFunctionType.Ln)
                        ot = dp.tile([128, CL], dt.float32, name="fot", tag="fot", bufs=3)
                        nc.vector.tensor_scalar(ot[:], lg[:], mx[:], ln[:],
                                                mybir.AluOpType.add,
                                                mybir.AluOpType.subtract)
                        nc.sync.dma_start(y_out[m * 128:(m + 1) * 128, :], ot[:])

    nc.compile()
    return nc


# ------------------------------------------------------------------- driver

def run(cfg, inputs, trace=False, mode="hw", MAXC=5):
    in_maps, segs, TOT = preprocess(cfg, **inputs)
    nc = build_nc(cfg, segs, TOT, MAXC=MAXC)
    outs = np.zeros((cfg.N, cfg.CL), np.float32)
    if mode == "sim":
        from concourse.bass_interp import MultiCoreSim
        sim = MultiCoreSim(nc, num_cores=cfg.NC, trace=False)
        for c, core in enumerate(sim.cores.values()):
            for k, v in in_maps[c].items():
                core.tensor(k)[:] = v
        sim.simulate()
        for c, core in enumerate(sim.cores.values()):
            outs[c * cfg.RPC_RAW:(c + 1) * cfg.RPC_RAW] = \
                np.asarray(core.tensor("y_out"))[:cfg.RPC_RAW]
        return outs, None
    from concourse import bass_utils
    res = bass_utils.run_bass_kernel_spmd(
        nc, in_maps, core_ids=list(range(cfg.NC)), trace=trace)
    for c in range(cfg.NC):
        outs[c * cfg.RPC_RAW:(c + 1) * cfg.RPC_RAW] = \
            res.results[c]["y_out"][:cfg.RPC_RAW]
    return outs, res


def kernel(**inputs):
    inputs = {k: np.asarray(v) for k, v in inputs.items()}
    out, _ = run(FULL, inputs, trace=False)
    return out



# revision 6
# speedup vs baseline: 2.5252x; 2.5252x over previous
"""MixHop GNN (nn_MixHopNetwork_75299366633514) on 8 TRN2 NeuronCores.

Strategy (self-contained; shapes hardcoded for the full problem):
  - Nodes sharded contiguously: core c owns rows [c*12500, (c+1)*12500),
    padded to 12800 local rows (100 blocks of 128).
  - Restructured math:
      r_i = relu(X @ Wu_i + bu_i)                       (dense, bf16)
      hop1: [A r1 | A r2]     = A @ [r1 | r2]           (512-wide spmm)
      hop2: [A^2 r2]          = A @ (A r2)              (256-wide spmm)
      abs1 = [r0 | A r1 | A^2 r2]                       (768 cols, 256-padded)
      B_i  = abs1 @ Wb_i                                (dense)
      hop3: [A B1 | A B2], hop4: [A^2 B2]
      abs2 = [B0 | A B1 | A^2 B2]   (biases bb folded into fc_b on host)
      out  = log_softmax(abs2 @ fc_w + fc_b')
  - spmm per core: edges sharded by destination; per destination-block (128
    rows) and source-window (int16 index range), ONE dma_gather per (block,
    window) pulls source rows from AllGathered fp8 operand buffers (HBM); a
    one-hot(val*32, fp8) matrix built by two DVE ops turns each 128-edge
    chunk into a PE matmul accumulating into PSUM; emits de-scale by 1/32.
  - Gather descriptor generation is the bottleneck: it runs on Q7 core pairs
    selected by the SWDGE queue number, so the 4 windows use queues 0-3 to
    engage all 8 Q7 cores in parallel.
  - AllGathers are split into 4 row-quarters, each issued as soon as the
    producing phase finishes that quarter (overlap with compute).
"""

import math
import numpy as np
import ml_dtypes

bf16 = ml_dtypes.bfloat16
f8e4 = ml_dtypes.float8_e4m3  # IEEE-style e4m3, max 240 — matches TRN FP8_EXP4
VAL_SCALE = 32.0
NQUEUES = 4  # SWDGE queues (1-4); queue q is served by Q7 cores 2q, 2q+1


class Cfg:
    def __init__(self, N, F, H, HP, CL, rpc_raw, rpc, qrows, deg_scale=None):
        self.NC = 8
        self.N = N            # real nodes
        self.F = F            # input features (multiple of 128)
        self.H = H            # real hidden per power
        self.HP = HP          # padded hidden (multiple of 128, >=H)
        self.W2 = 2 * HP      # fused double width
        self.CL = CL          # classes
        self.RPC_RAW = rpc_raw  # real rows per core
        self.RPC = rpc        # padded rows per core (multiple of 512)
        self.QROWS = qrows    # rows per quarter (RPC/4, multiple of 128)
        self.NBLK = rpc // 128
        self.NWIN = 4
        self.WIN_ROWS = self.NC * qrows  # rows per AG window buffer
        assert rpc == 4 * qrows and qrows % 128 == 0
        assert self.WIN_ROWS <= 32768


FULL = Cfg(N=100000, F=512, H=200, HP=256, CL=40,
           rpc_raw=12500, rpc=12800, qrows=3200)


# ---------------------------------------------------------------- host side

def preprocess(cfg, features, adj_row, adj_col, adj_val, Wu, bu, Wb, bb, fc_w, fc_b):
    """Pure-numpy preprocessing -> per-core in_maps + shared segment table."""
    NC, RPCR, RPC, QROWS = cfg.NC, cfg.RPC_RAW, cfg.RPC, cfg.QROWS
    NBLK, NWIN = cfg.NBLK, cfg.NWIN
    H, HP, F, CL = cfg.H, cfg.HP, cfg.F, cfg.CL

    row = adj_row.astype(np.int64)
    col = adj_col.astype(np.int64)
    val = adj_val.astype(np.float32)

    core = row // RPCR
    dl = row - core * RPCR           # local dest row 0..RPCR-1
    b = dl >> 7                      # dest block
    doff = dl & 127
    cs = col // RPCR
    ls = col - cs * RPCR
    w = ls // QROWS                  # source window
    idx = cs * QROWS + (ls - w * QROWS)  # row within window buffer

    # counts per (core, b, w)
    key = (core * NBLK + b) * NWIN + w
    counts = np.bincount(key, minlength=NC * NBLK * NWIN).reshape(NC, NBLK, NWIN)
    Cs = np.maximum(1, -(-counts.max(axis=0) // 128))  # [NBLK, NWIN] chunks, >=1
    seg_off = np.zeros((NBLK, NWIN), np.int64)         # chunk offsets
    flat = Cs.reshape(-1)
    seg_off.reshape(-1)[1:] = np.cumsum(flat)[:-1]
    TOT = int(flat.sum()) * 128                        # padded slots

    # slot of each edge: seg_off[b,w]*128 + rank within its (core,b,w) group
    order = np.lexsort((w, b, core))
    so_r, so_b, so_w = core[order], b[order], w[order]
    gkey = (so_r * NBLK + so_b) * NWIN + so_w
    grp_start = np.zeros(len(gkey), np.int64)
    new_grp = np.ones(len(gkey), bool)
    new_grp[1:] = gkey[1:] != gkey[:-1]
    starts_idx = np.nonzero(new_grp)[0]
    grp_start[starts_idx] = starts_idx
    grp_start = np.maximum.accumulate(grp_start)
    cum = np.arange(len(gkey)) - grp_start
    slot = seg_off[so_b, so_w] * 128 + cum

    # build padded per-core meta arrays
    idx_pad = np.zeros((NC, TOT), np.int16)
    doff_pad = np.zeros((NC, TOT), np.int16)
    val_pad = np.zeros((NC, TOT), np.float32)
    idx_pad[so_r, slot] = idx[order].astype(np.int16)
    doff_pad[so_r, slot] = doff[order].astype(np.int16)
    val_pad[so_r, slot] = val[order]

    # device layouts
    idx_dev = np.zeros((NC, 128, TOT // 16), np.int16)
    blk = np.transpose(idx_pad.reshape(NC, TOT // 16, 16), (0, 2, 1))  # [NC,16,TOT/16]
    for k in range(8):
        idx_dev[:, 16 * k:16 * (k + 1), :] = blk
    doff_dev = np.ascontiguousarray(
        np.transpose(doff_pad.reshape(NC, TOT // 128, 128), (0, 2, 1))
    ).astype(bf16)
    val_dev = np.ascontiguousarray(
        np.transpose((val_pad * VAL_SCALE).reshape(NC, TOT // 128, 128), (0, 2, 1))
    ).astype(f8e4)

    # features: per-core [F, RPC] bf16 (transposed, padded)
    featsT = np.zeros((NC, F, RPC), bf16)
    for c in range(NC):
        featsT[c, :, :RPCR] = features[c * RPCR:(c + 1) * RPCR].T.astype(bf16)

    # weights, padded
    Wu_p = np.zeros((3, F, HP), bf16)
    Wu_p[:, :, :H] = Wu.astype(bf16)
    bu_rep = np.zeros((3, 128, HP), np.float32)
    bu_rep[:, :, :H] = bu[:, None, :]
    # Wb rows: [0:H]->0:H, [H:2H]->HP:HP+H, [2H:3H]->2HP:2HP+H
    Wb_p = np.zeros((3, 3 * HP, HP), bf16)
    fcw_p = np.zeros((3 * HP, CL), bf16)
    for j in range(3):
        Wb_p[:, j * HP:j * HP + H, :H] = Wb[:, j * H:(j + 1) * H, :].astype(bf16)
        fcw_p[j * HP:j * HP + H, :] = fc_w[j * H:(j + 1) * H, :].astype(bf16)
    # fold the bottom biases into the final-layer bias:
    # (h_i + bb_i) @ fcw_i = h_i @ fcw_i + bb_i @ fcw_i
    fcb_eff = fc_b.astype(np.float32).copy()
    for i in range(3):
        fcb_eff += bb[i].astype(np.float32) @ fc_w[i * H:(i + 1) * H].astype(np.float32)
    fcb_rep = np.broadcast_to(fcb_eff, (128, CL)).copy()
    iota = np.broadcast_to(np.arange(128, dtype=np.float32), (128, 128)).astype(bf16)

    in_maps = []
    for c in range(NC):
        in_maps.append({
            "featsT": featsT[c],
            "meta_idx": idx_dev[c],
            "meta_doff": doff_dev[c],
            "meta_val": val_dev[c],
            "Wu_p": Wu_p, "bu_rep": bu_rep.astype(np.float32),
            "Wb_p": Wb_p,
            "fcw_p": fcw_p, "fcb_rep": fcb_rep,
            "iota": np.asarray(iota),
        })
    segs = [[(int(seg_off[bb_, ww]), int(Cs[bb_, ww])) for ww in range(NWIN)]
            for bb_ in range(NBLK)]
    return in_maps, segs, TOT


# -------------------------------------------------------------- device side

def build_nc(cfg, segs, TOT, MAXC=7):
    import concourse.bacc as bacc
    import concourse.mybir as mybir
    import concourse.tile as tile

    dt = mybir.dt
    f8 = dt.float8e4
    NC, RPC, QROWS = cfg.NC, cfg.RPC, cfg.QROWS
    NBLK, NWIN, WIN_ROWS = cfg.NBLK, cfg.NWIN, cfg.WIN_ROWS
    F, HP, W2, CL = cfg.F, cfg.HP, cfg.W2, cfg.CL
    KF = F // 128          # feature k-chunks
    KA = 3 * HP // 128     # abstract k-chunks
    MG = 10 if NBLK % 10 == 0 else 2   # m-tiles per group
    NG = NBLK // MG
    BPQ = QROWS // 128     # blocks per AG quarter
    ALL = list(range(NC))
    DESCALE = 1.0 / VAL_SCALE

    nc = bacc.Bacc("TRN2", target_bir_lowering=False, debug=False,
                   num_devices=NC, num_swdge_queues=NQUEUES)

    featsT = nc.dram_tensor("featsT", [F, RPC], dt.bfloat16, kind="ExternalInput")
    meta_idx = nc.dram_tensor("meta_idx", [128, TOT // 16], dt.int16, kind="ExternalInput")
    meta_doff = nc.dram_tensor("meta_doff", [128, TOT // 128], dt.bfloat16, kind="ExternalInput")
    meta_val = nc.dram_tensor("meta_val", [128, TOT // 128], f8, kind="ExternalInput")
    Wu_p = nc.dram_tensor("Wu_p", [3, F, HP], dt.bfloat16, kind="ExternalInput")
    bu_rep = nc.dram_tensor("bu_rep", [3, 128, HP], dt.float32, kind="ExternalInput")
    Wb_p = nc.dram_tensor("Wb_p", [3, 3 * HP, HP], dt.bfloat16, kind="ExternalInput")
    fcw_p = nc.dram_tensor("fcw_p", [3 * HP, CL], dt.bfloat16, kind="ExternalInput")
    fcb_rep = nc.dram_tensor("fcb_rep", [128, CL], dt.float32, kind="ExternalInput")
    iota_in = nc.dram_tensor("iota", [128, 128], dt.bfloat16, kind="ExternalInput")
    y_out = nc.dram_tensor("y_out", [RPC, CL], dt.float32, kind="ExternalOutput")

    with tile.TileContext(nc) as tc:
        with (
            tc.tile_pool(name="const", bufs=1) as cpool,
            tc.tile_pool(name="dram", bufs=1, space="DRAM") as dram,
        ):
            # ---- resident constants
            iota_t = cpool.tile([128, 128], dt.bfloat16)
            nc.sync.dma_start(iota_t[:], iota_in[:])
            idx_t = cpool.tile([128, TOT // 16], dt.int16)
            nc.sync.dma_start(idx_t[:], meta_idx[:])
            doff_t = cpool.tile([128, TOT // 128], dt.bfloat16)
            nc.sync.dma_start(doff_t[:], meta_doff[:])
            val_t = cpool.tile([128, TOT // 128], f8)
            nc.sync.dma_start(val_t[:], meta_val[:])
            wu_t = cpool.tile([128, 3, KF, HP], dt.bfloat16)
            for i in range(3):
                for kc in range(KF):
                    nc.sync.dma_start(wu_t[:, i, kc, :], Wu_p[i, kc * 128:(kc + 1) * 128, :])
            wb_t = cpool.tile([128, 3, KA, HP], dt.bfloat16)
            for i in range(3):
                for kc in range(KA):
                    nc.sync.dma_start(wb_t[:, i, kc, :], Wb_p[i, kc * 128:(kc + 1) * 128, :])
            fcw_t = cpool.tile([128, KA, CL], dt.bfloat16)
            for kc in range(KA):
                nc.sync.dma_start(fcw_t[:, kc, :], fcw_p[kc * 128:(kc + 1) * 128, :])
            bu_t = cpool.tile([128, 3, HP], dt.float32)
            for i in range(3):
                nc.sync.dma_start(bu_t[:, i, :], bu_rep[i])
            fcb_t = cpool.tile([128, CL], dt.float32)
            nc.sync.dma_start(fcb_t[:], fcb_rep[:])

            # ---- DRAM intermediates (hop operands in fp8)
            abs1 = dram.tile([RPC, 3 * HP], dt.bfloat16)
            abs2 = dram.tile([RPC, 3 * HP], dt.bfloat16)
            agin1 = [dram.tile([QROWS, W2], f8, name=f"agin1_{q}") for q in range(4)]
            agbuf1 = [dram.tile([WIN_ROWS, W2], f8, addr_space="Shared", name=f"agbuf1_{q}") for q in range(4)]
            agin2 = [dram.tile([QROWS, HP], f8, name=f"agin2_{q}") for q in range(4)]
            agbuf2 = [dram.tile([WIN_ROWS, HP], f8, addr_space="Shared", name=f"agbuf2_{q}") for q in range(4)]
            agin3 = [dram.tile([QROWS, W2], f8, name=f"agin3_{q}") for q in range(4)]
            agbuf3 = [dram.tile([WIN_ROWS, W2], f8, addr_space="Shared", name=f"agbuf3_{q}") for q in range(4)]
            agin4 = [dram.tile([QROWS, HP], f8, name=f"agin4_{q}") for q in range(4)]
            agbuf4 = [dram.tile([WIN_ROWS, HP], f8, addr_space="Shared", name=f"agbuf4_{q}") for q in range(4)]

            def allgather_q(agin_q, agbuf_q, q):
                nc.gpsimd.collective_compute(
                    "AllGather", mybir.AluOpType.bypass,
                    replica_groups=[ALL],
                    ins=[agin_q[q][:].opt()],
                    outs=[agbuf_q[q][:].opt()],
                )

            # ================= D1: r_i = relu(X @ Wu_i + bu_i)
            with (
                tc.tile_pool(name="d1", bufs=2) as dp,
                tc.tile_pool(name="d1ps", bufs=2, space="PSUM") as pp,
            ):
                for g in range(NG):
                    xt = dp.tile([128, KF, MG * 128], dt.bfloat16, name="xt", tag="xt", bufs=2)
                    for kc in range(KF):
                        nc.sync.dma_start(
                            xt[:, kc, :],
                            featsT[kc * 128:(kc + 1) * 128, g * MG * 128:(g + 1) * MG * 128])
                    for ml in range(MG):
                        m = g * MG + ml
                        ps = pp.tile([128, 3, HP], dt.float32, name="d1p", tag="d1p", bufs=2)
                        for i in range(3):
                            for kc in range(KF):
                                nc.tensor.matmul(
                                    ps[:, i, :],
                                    xt[:, kc, ml * 128:(ml + 1) * 128],
                                    wu_t[:, i, kc, :],
                                    start=(kc == 0), stop=(kc == KF - 1))
                        rt = dp.tile([128, 3, HP], dt.float32, name="rt", tag="rt", bufs=3)
                        r0 = dp.tile([128, HP], dt.bfloat16, name="r0", tag="r0", bufs=3)
                        rf = dp.tile([128, 2, HP], f8, name="rf", tag="rf", bufs=3)
                        for i in range(3):
                            nc.vector.tensor_add(rt[:, i, :], ps[:, i, :], bu_t[:, i, :])
                        nc.vector.tensor_relu(r0[:], rt[:, 0, :])
                        nc.vector.tensor_relu(rf[:, 0, :], rt[:, 1, :])
                        nc.vector.tensor_relu(rf[:, 1, :], rt[:, 2, :])
                        q, lr = divmod(m * 128, QROWS)
                        nc.sync.dma_start(abs1[m * 128:(m + 1) * 128, 0:HP], r0[:])
                        nc.sync.dma_start(agin1[q][lr:lr + 128, 0:HP], rf[:, 0, :])
                        nc.sync.dma_start(agin1[q][lr:lr + 128, HP:W2], rf[:, 1, :])
                        if (m + 1) % BPQ == 0:
                            allgather_q(agin1, agbuf1, q)

            # ================= hop phases
            def hop(agbuf_q, width, emits, next_ag=None):
                """emits: list of (col0, dst_kind, dst, dcol)
                   dst_kind: 'abs' -> dst[rows, dcol:dcol+HP] (bf16)
                             'ag'  -> dst[q][lr:lr+128, dcol:dcol+HP] (fp8)
                   next_ag: (agin_q, agbuf_q) to issue per-quarter after the
                   producing blocks complete."""
                with (
                    tc.tile_pool(name="hp", bufs=2) as hp,
                    tc.tile_pool(name="hps", bufs=4, space="PSUM") as hpp,
                ):
                    for b in range(NBLK):
                        bw = segs[b]
                        Cb = sum(C for (_, C) in bw)
                        gt = hp.tile([128, Cb, width], f8, name="gt", tag="gt",
                                     bufs=3)
                        co = 0
                        for w in range(NWIN):
                            off, C = bw[w]
                            while C > 0:
                                Cg = min(C, MAXC)
                                nc.gpsimd.dma_gather(
                                    gt[:, co:co + Cg, :],
                                    agbuf_q[w][:],
                                    idx_t[:, off * 8:(off + Cg) * 8],
                                    num_idxs=Cg * 128,
                                    num_idxs_reg=Cg * 128,
                                    elem_size=width,
                                    queue_num=w % NQUEUES)
                                co += Cg
                                off += Cg
                                C -= Cg
                        oh = hp.tile([128, Cb, 128], f8, name="oh", tag="oh",
                                     bufs=2)
                        c0 = bw[0][0]
                        nc.vector.tensor_tensor(
                            oh[:],
                            iota_t[:].unsqueeze(1).broadcast_to([128, Cb, 128]),
                            doff_t[:, c0:c0 + Cb].unsqueeze(2).broadcast_to([128, Cb, 128]),
                            mybir.AluOpType.is_equal)
                        nc.vector.tensor_tensor(
                            oh[:], oh[:],
                            val_t[:, c0:c0 + Cb].unsqueeze(2).broadcast_to([128, Cb, 128]),
                            mybir.AluOpType.mult)
                        ps = hpp.tile([128, width], dt.float32, name="hpsum", tag="hpsum",
                                      bufs=4)
                        for c in range(Cb):
                            nc.tensor.matmul(ps[:], oh[:, c, :], gt[:, c, :],
                                             start=(c == 0), stop=(c == Cb - 1))
                        q, lr = divmod(b * 128, QROWS)
                        for (col0, kind, dst, dcol) in emits:
                            if kind == 'abs':
                                ot = hp.tile([128, HP], dt.bfloat16, name="ot", tag="ot", bufs=4)
                                nc.scalar.activation(ot[:], ps[:, col0:col0 + HP],
                                                     mybir.ActivationFunctionType.Copy,
                                                     scale=DESCALE)
                                nc.sync.dma_start(dst[b * 128:(b + 1) * 128, dcol:dcol + HP], ot[:])
                            else:
                                of = hp.tile([128, HP], f8, name="of", tag="of", bufs=4)
                                nc.scalar.activation(of[:], ps[:, col0:col0 + HP],
                                                     mybir.ActivationFunctionType.Copy,
                                                     scale=DESCALE)
                                nc.sync.dma_start(dst[q][lr:lr + 128, dcol:dcol + HP], of[:])
                        if next_ag is not None and (b + 1) % BPQ == 0:
                            allgather_q(next_ag[0], next_ag[1], q)

            # H1: A[r1|r2] -> Ar1 to abs1 cols HP:2HP ; Ar2 to agin2
            hop(agbuf1, W2, [(0, 'abs', abs1, HP), (HP, 'ag', agin2, 0)],
                next_ag=(agin2, agbuf2))
            # H2: A^2 r2 -> abs1 cols 2HP:3HP
            hop(agbuf2, HP, [(0, 'abs', abs1, 2 * HP)])

            # ================= D2: B_i = abs1 @ Wb_i
            with (
                tc.tile_pool(name="d2", bufs=2) as dp,
                tc.tile_pool(name="d2ps", bufs=2, space="PSUM") as pp,
            ):
                for g in range(NG):
                    a1t = dp.tile([128, KA, MG * 128], dt.bfloat16, name="a1t", tag="a1t", bufs=2)
                    for kc in range(KA):
                        nc.sync.dma_start(
                            a1t[:, kc, :],
                            abs1[g * MG * 128:(g + 1) * MG * 128, kc * 128:(kc + 1) * 128],
                            transpose=True)
                    for ml in range(MG):
                        m = g * MG + ml
                        ps = pp.tile([128, 3, HP], dt.float32, name="d2p", tag="d2p", bufs=2)
                        for i in range(3):
                            for kc in range(KA):
                                nc.tensor.matmul(
                                    ps[:, i, :],
                                    a1t[:, kc, ml * 128:(ml + 1) * 128],
                                    wb_t[:, i, kc, :],
                                    start=(kc == 0), stop=(kc == KA - 1))
                        b0 = dp.tile([128, HP], dt.bfloat16, name="b0", tag="b0", bufs=3)
                        bff = dp.tile([128, 2, HP], f8, name="bff", tag="bff", bufs=3)
                        nc.scalar.activation(b0[:], ps[:, 0, :],
                                             mybir.ActivationFunctionType.Copy)
                        nc.scalar.activation(bff[:, 0, :], ps[:, 1, :],
                                             mybir.ActivationFunctionType.Copy)
                        nc.scalar.activation(bff[:, 1, :], ps[:, 2, :],
                                             mybir.ActivationFunctionType.Copy)
                        q, lr = divmod(m * 128, QROWS)
                        nc.sync.dma_start(abs2[m * 128:(m + 1) * 128, 0:HP], b0[:])
                        nc.sync.dma_start(agin3[q][lr:lr + 128, 0:HP], bff[:, 0, :])
                        nc.sync.dma_start(agin3[q][lr:lr + 128, HP:W2], bff[:, 1, :])
                        if (m + 1) % BPQ == 0:
                            allgather_q(agin3, agbuf3, q)

            # H3: A[B1|B2] -> AB1 to abs2 ; AB2 to agin4
            hop(agbuf3, W2, [(0, 'abs', abs2, HP), (HP, 'ag', agin4, 0)],
                next_ag=(agin4, agbuf4))
            # H4: A^2 B2 -> abs2 cols 2HP:3HP
            hop(agbuf4, HP, [(0, 'abs', abs2, 2 * HP)])

            # ================= D3: logits + log_softmax
            with (
                tc.tile_pool(name="d3", bufs=2) as dp,
                tc.tile_pool(name="d3ps", bufs=2, space="PSUM") as pp,
            ):
                for g in range(NG):
                    a2t = dp.tile([128, KA, MG * 128], dt.bfloat16, name="a2t", tag="a2t", bufs=2)
                    for kc in range(KA):
                        nc.sync.dma_start(
                            a2t[:, kc, :],
                            abs2[g * MG * 128:(g + 1) * MG * 128, kc * 128:(kc + 1) * 128],
                            transpose=True)
                    for ml in range(MG):
                        m = g * MG + ml
                        ps = pp.tile([128, CL], dt.float32, name="d3p", tag="d3p", bufs=2)
                        for kc in range(KA):
                            nc.tensor.matmul(
                                ps[:],
                                a2t[:, kc, ml * 128:(ml + 1) * 128],
                                fcw_t[:, kc, :],
                                start=(kc == 0), stop=(kc == KA - 1))
                        lg = dp.tile([128, CL], dt.float32, name="lg", tag="lg", bufs=3)
                        nc.vector.tensor_add(lg[:], ps[:], fcb_t[:])
                        mx = dp.tile([128, 1], dt.float32, name="mx", tag="mx", bufs=3)
                        nc.vector.tensor_reduce(mx[:], lg[:], mybir.AxisListType.X,
                                                mybir.AluOpType.max, negate=True)
                        ex = dp.tile([128, CL], dt.float32, name="ex", tag="ex", bufs=3)
                        sm = dp.tile([128, 1], dt.float32, name="sm", tag="sm", bufs=3)
                        nc.scalar.activation(ex[:], lg[:], mybir.ActivationFunctionType.Exp,
                                             bias=mx[:], accum_out=sm[:])
                        ln = dp.tile([128, 1], dt.float32, name="ln", tag="ln", bufs=3)
                        nc.scalar.activation(ln[:], sm[:], mybir.ActivationFunctionType.Ln)
                        ot = dp.tile([128, CL], dt.float32, name="fot", tag="fot", bufs=3)
                        nc.vector.tensor_scalar(ot[:], lg[:], mx[:], ln[:],
                                                mybir.AluOpType.add,
                                                mybir.AluOpType.subtract)
                        nc.sync.dma_start(y_out[m * 128:(m + 1) * 128, :], ot[:])

    nc.compile()
    return nc


# ------------------------------------------------------------------- driver

def run(cfg, inputs, trace=False, mode="hw", MAXC=7):
    in_maps, segs, TOT = preprocess(cfg, **inputs)
    nc = build_nc(cfg, segs, TOT, MAXC=MAXC)
    outs = np.zeros((cfg.N, cfg.CL), np.float32)
    if mode == "sim":
        from concourse.bass_interp import MultiCoreSim
        sim = MultiCoreSim(nc, num_cores=cfg.NC, trace=False)
        for c, core in enumerate(sim.cores.values()):
            for k, v in in_maps[c].items():
                core.tensor(k)[:] = v
        sim.simulate()
        for c, core in enumerate(sim.cores.values()):
            outs[c * cfg.RPC_RAW:(c + 1) * cfg.RPC_RAW] = \
                np.asarray(core.tensor("y_out"))[:cfg.RPC_RAW]
        return outs, None
    from concourse import bass_utils
    res = bass_utils.run_bass_kernel_spmd(
        nc, in_maps, core_ids=list(range(cfg.NC)), trace=trace)
    for c in range(cfg.NC):
        outs[c * cfg.RPC_RAW:(c + 1) * cfg.RPC_RAW] = \
            res.results[c]["y_out"][:cfg.RPC_RAW]
    return outs, res


def kernel(**inputs):
    inputs = {k: np.asarray(v) for k, v in inputs.items()}
    out, _ = run(FULL, inputs, trace=False)
    return out


# revision 10
# speedup vs baseline: 2.8448x; 1.1266x over previous
"""MixHop GNN (nn_MixHopNetwork_75299366633514) on 8 TRN2 NeuronCores.

Strategy (self-contained; shapes hardcoded for the full problem):
  - Nodes sharded contiguously: core c owns rows [c*12500, (c+1)*12500),
    padded to 12800 local rows (100 blocks of 128).
  - Restructured math:
      r_i = relu(X @ Wu_i + bu_i)                       (dense, bf16)
      hop1: [A r1 | A r2]     = A @ [r1 | r2]           (512-wide spmm)
      hop2: [A^2 r2]          = A @ (A r2)              (256-wide spmm)
      abs1 = [r0 | A r1 | A^2 r2]                       (768 cols, 256-padded)
      B_i  = abs1 @ Wb_i                                (dense)
      hop3: [A B1 | A B2], hop4: [A^2 B2]
      abs2 = [B0 | A B1 | A^2 B2]   (biases bb folded into fc_b on host)
      out  = log_softmax(abs2 @ fc_w + fc_b')
  - spmm per core: edges sharded by destination; per destination-block (128
    rows) and source-window (int16 index range), ONE dma_gather per (block,
    window) pulls source rows from AllGathered fp8 operand buffers (HBM); a
    one-hot(val*32, fp8) matrix built by two DVE ops turns each 128-edge
    chunk into a PE matmul accumulating into PSUM; emits de-scale by 1/32.
  - Gather descriptor generation is the bottleneck: it runs on Q7 core pairs
    selected by the SWDGE queue number, so the 4 windows use queues 0-3 to
    engage all 8 Q7 cores in parallel.
  - AllGathers are split into 4 row-quarters, each issued as soon as the
    producing phase finishes that quarter (overlap with compute).
"""

import math
import numpy as np
import ml_dtypes

bf16 = ml_dtypes.bfloat16
f8e4 = ml_dtypes.float8_e4m3  # IEEE-style e4m3, max 240 — matches TRN FP8_EXP4
VAL_SCALE = 32.0
NQUEUES = 4  # SWDGE queues (1-4); queue q is served by Q7 cores 2q, 2q+1


class Cfg:
    def __init__(self, N, F, H, HP, CL, rpc_raw, rpc, qrows, deg_scale=None):
        self.NC = 8
        self.N = N            # real nodes
        self.F = F            # input features (multiple of 128)
        self.H = H            # real hidden per power
        self.HP = HP          # padded hidden (multiple of 128, >=H)
        self.W2 = 2 * HP      # fused double width
        self.CL = CL          # classes
        self.RPC_RAW = rpc_raw  # real rows per core
        self.RPC = rpc        # padded rows per core (multiple of 512)
        self.QROWS = qrows    # rows per quarter (RPC/4, multiple of 128)
        self.NBLK = rpc // 128
        self.NWIN = 4
        self.WIN_ROWS = self.NC * qrows  # rows per AG window buffer
        assert rpc == 4 * qrows and qrows % 128 == 0
        assert self.WIN_ROWS <= 32768


FULL = Cfg(N=100000, F=512, H=200, HP=256, CL=40,
           rpc_raw=12500, rpc=12800, qrows=3200)


# ---------------------------------------------------------------- host side

def preprocess(cfg, features, adj_row, adj_col, adj_val, Wu, bu, Wb, bb, fc_w, fc_b):
    """Pure-numpy preprocessing -> per-core in_maps + shared segment table."""
    NC, RPCR, RPC, QROWS = cfg.NC, cfg.RPC_RAW, cfg.RPC, cfg.QROWS
    NBLK, NWIN = cfg.NBLK, cfg.NWIN
    H, HP, F, CL = cfg.H, cfg.HP, cfg.F, cfg.CL

    row = adj_row.astype(np.int64)
    col = adj_col.astype(np.int64)
    val = adj_val.astype(np.float32)

    core = row // RPCR
    dl = row - core * RPCR           # local dest row 0..RPCR-1
    b = dl >> 7                      # dest block
    doff = dl & 127
    cs = col // RPCR
    ls = col - cs * RPCR
    w = ls // QROWS                  # source window
    idx = cs * QROWS + (ls - w * QROWS)  # row within window buffer

    # counts per (core, b, w)
    key = (core * NBLK + b) * NWIN + w
    counts = np.bincount(key, minlength=NC * NBLK * NWIN).reshape(NC, NBLK, NWIN)
    Cs = np.maximum(1, -(-counts.max(axis=0) // 128))  # [NBLK, NWIN] chunks, >=1
    seg_off = np.zeros((NBLK, NWIN), np.int64)         # chunk offsets
    flat = Cs.reshape(-1)
    seg_off.reshape(-1)[1:] = np.cumsum(flat)[:-1]
    TOT = int(flat.sum()) * 128                        # padded slots

    # slot of each edge: seg_off[b,w]*128 + rank within its (core,b,w) group
    order = np.lexsort((w, b, core))
    so_r, so_b, so_w = core[order], b[order], w[order]
    gkey = (so_r * NBLK + so_b) * NWIN + so_w
    grp_start = np.zeros(len(gkey), np.int64)
    new_grp = np.ones(len(gkey), bool)
    new_grp[1:] = gkey[1:] != gkey[:-1]
    starts_idx = np.nonzero(new_grp)[0]
    grp_start[starts_idx] = starts_idx
    grp_start = np.maximum.accumulate(grp_start)
    cum = np.arange(len(gkey)) - grp_start
    slot = seg_off[so_b, so_w] * 128 + cum

    # build padded per-core meta arrays
    idx_pad = np.zeros((NC, TOT), np.int16)
    doff_pad = np.zeros((NC, TOT), np.int16)
    val_pad = np.zeros((NC, TOT), np.float32)
    idx_pad[so_r, slot] = idx[order].astype(np.int16)
    doff_pad[so_r, slot] = doff[order].astype(np.int16)
    val_pad[so_r, slot] = val[order]

    # device layouts
    idx_dev = np.zeros((NC, 128, TOT // 16), np.int16)
    blk = np.transpose(idx_pad.reshape(NC, TOT // 16, 16), (0, 2, 1))  # [NC,16,TOT/16]
    for k in range(8):
        idx_dev[:, 16 * k:16 * (k + 1), :] = blk
    doff_dev = np.ascontiguousarray(
        np.transpose(doff_pad.reshape(NC, TOT // 128, 128), (0, 2, 1))
    ).astype(bf16)
    val_dev = np.ascontiguousarray(
        np.transpose((val_pad * VAL_SCALE).reshape(NC, TOT // 128, 128), (0, 2, 1))
    ).astype(f8e4)

    # features: per-core [F, RPC] bf16 (transposed, padded)
    featsT = np.zeros((NC, F, RPC), bf16)
    for c in range(NC):
        featsT[c, :, :RPCR] = features[c * RPCR:(c + 1) * RPCR].T.astype(bf16)

    # weights, padded
    Wu_p = np.zeros((3, F, HP), bf16)
    Wu_p[:, :, :H] = Wu.astype(bf16)
    bu_rep = np.zeros((3, 128, HP), np.float32)
    bu_rep[:, :, :H] = bu[:, None, :]
    # Wb rows: [0:H]->0:H, [H:2H]->HP:HP+H, [2H:3H]->2HP:2HP+H
    Wb_p = np.zeros((3, 3 * HP, HP), bf16)
    fcw_p = np.zeros((3 * HP, CL), bf16)
    for j in range(3):
        Wb_p[:, j * HP:j * HP + H, :H] = Wb[:, j * H:(j + 1) * H, :].astype(bf16)
        fcw_p[j * HP:j * HP + H, :] = fc_w[j * H:(j + 1) * H, :].astype(bf16)
    # fold the bottom biases into the final-layer bias:
    # (h_i + bb_i) @ fcw_i = h_i @ fcw_i + bb_i @ fcw_i
    fcb_eff = fc_b.astype(np.float32).copy()
    for i in range(3):
        fcb_eff += bb[i].astype(np.float32) @ fc_w[i * H:(i + 1) * H].astype(np.float32)
    fcb_rep = np.broadcast_to(fcb_eff, (128, CL)).copy()
    iota = np.broadcast_to(np.arange(128, dtype=np.float32), (128, 128)).astype(bf16)

    in_maps = []
    for c in range(NC):
        in_maps.append({
            "featsT": featsT[c],
            "meta_idx": idx_dev[c],
            "meta_doff": doff_dev[c],
            "meta_val": val_dev[c],
            "Wu_p": Wu_p, "bu_rep": bu_rep.astype(np.float32),
            "Wb_p": Wb_p,
            "fcw_p": fcw_p, "fcb_rep": fcb_rep,
            "iota": np.asarray(iota),
        })
    segs = [[(int(seg_off[bb_, ww]), int(Cs[bb_, ww])) for ww in range(NWIN)]
            for bb_ in range(NBLK)]
    return in_maps, segs, TOT


# -------------------------------------------------------------- device side

def build_nc(cfg, segs, TOT, MAXC=16):
    import concourse.bacc as bacc
    import concourse.mybir as mybir
    import concourse.tile as tile

    dt = mybir.dt
    f8 = dt.float8e4
    NC, RPC, QROWS = cfg.NC, cfg.RPC, cfg.QROWS
    NBLK, NWIN, WIN_ROWS = cfg.NBLK, cfg.NWIN, cfg.WIN_ROWS
    F, HP, W2, CL = cfg.F, cfg.HP, cfg.W2, cfg.CL
    KF = F // 128          # feature k-chunks
    KA = 3 * HP // 128     # abstract k-chunks
    MG = 10 if NBLK % 10 == 0 else 2   # m-tiles per group
    NG = NBLK // MG
    BPQ = QROWS // 128     # blocks per AG quarter
    ALL = list(range(NC))
    DESCALE = 1.0 / VAL_SCALE

    nc = bacc.Bacc("TRN2", target_bir_lowering=False, debug=False,
                   num_devices=NC, num_swdge_queues=NQUEUES)

    featsT = nc.dram_tensor("featsT", [F, RPC], dt.bfloat16, kind="ExternalInput")
    meta_idx = nc.dram_tensor("meta_idx", [128, TOT // 16], dt.int16, kind="ExternalInput")
    meta_doff = nc.dram_tensor("meta_doff", [128, TOT // 128], dt.bfloat16, kind="ExternalInput")
    meta_val = nc.dram_tensor("meta_val", [128, TOT // 128], f8, kind="ExternalInput")
    Wu_p = nc.dram_tensor("Wu_p", [3, F, HP], dt.bfloat16, kind="ExternalInput")
    bu_rep = nc.dram_tensor("bu_rep", [3, 128, HP], dt.float32, kind="ExternalInput")
    Wb_p = nc.dram_tensor("Wb_p", [3, 3 * HP, HP], dt.bfloat16, kind="ExternalInput")
    fcw_p = nc.dram_tensor("fcw_p", [3 * HP, CL], dt.bfloat16, kind="ExternalInput")
    fcb_rep = nc.dram_tensor("fcb_rep", [128, CL], dt.float32, kind="ExternalInput")
    iota_in = nc.dram_tensor("iota", [128, 128], dt.bfloat16, kind="ExternalInput")
    y_out = nc.dram_tensor("y_out", [RPC, CL], dt.float32, kind="ExternalOutput")

    with tile.TileContext(nc) as tc:
        with (
            tc.tile_pool(name="const", bufs=1) as cpool,
            tc.tile_pool(name="dram", bufs=1, space="DRAM") as dram,
        ):
            # ---- resident constants
            iota_t = cpool.tile([128, 128], dt.bfloat16)
            nc.sync.dma_start(iota_t[:], iota_in[:])
            idx_t = cpool.tile([128, TOT // 16], dt.int16)
            nc.sync.dma_start(idx_t[:], meta_idx[:])
            doff_t = cpool.tile([128, TOT // 128], dt.bfloat16)
            nc.sync.dma_start(doff_t[:], meta_doff[:])
            val_t = cpool.tile([128, TOT // 128], f8)
            nc.sync.dma_start(val_t[:], meta_val[:])
            wu_t = cpool.tile([128, 3, KF, HP], dt.bfloat16)
            for i in range(3):
                for kc in range(KF):
                    nc.sync.dma_start(wu_t[:, i, kc, :], Wu_p[i, kc * 128:(kc + 1) * 128, :])
            wb_t = cpool.tile([128, 3, KA, HP], dt.bfloat16)
            for i in range(3):
                for kc in range(KA):
                    nc.sync.dma_start(wb_t[:, i, kc, :], Wb_p[i, kc * 128:(kc + 1) * 128, :])
            fcw_t = cpool.tile([128, KA, CL], dt.bfloat16)
            for kc in range(KA):
                nc.sync.dma_start(fcw_t[:, kc, :], fcw_p[kc * 128:(kc + 1) * 128, :])
            bu_t = cpool.tile([128, 3, HP], dt.float32)
            for i in range(3):
                nc.sync.dma_start(bu_t[:, i, :], bu_rep[i])
            fcb_t = cpool.tile([128, CL], dt.float32)
            nc.sync.dma_start(fcb_t[:], fcb_rep[:])

            # ---- DRAM intermediates (hop operands in fp8)
            abs1 = dram.tile([RPC, 3 * HP], dt.bfloat16)
            abs2 = dram.tile([RPC, 3 * HP], dt.bfloat16)
            agin1 = [dram.tile([QROWS, W2], f8, name=f"agin1_{q}") for q in range(4)]
            agbuf1 = [dram.tile([WIN_ROWS, W2], f8, addr_space="Shared", name=f"agbuf1_{q}") for q in range(4)]
            agin2 = [dram.tile([QROWS, HP], f8, name=f"agin2_{q}") for q in range(4)]
            agbuf2 = [dram.tile([WIN_ROWS, HP], f8, addr_space="Shared", name=f"agbuf2_{q}") for q in range(4)]
            agin3 = [dram.tile([QROWS, W2], f8, name=f"agin3_{q}") for q in range(4)]
            agbuf3 = [dram.tile([WIN_ROWS, W2], f8, addr_space="Shared", name=f"agbuf3_{q}") for q in range(4)]
            agin4 = [dram.tile([QROWS, HP], f8, name=f"agin4_{q}") for q in range(4)]
            agbuf4 = [dram.tile([WIN_ROWS, HP], f8, addr_space="Shared", name=f"agbuf4_{q}") for q in range(4)]

            def allgather_q(agin_q, agbuf_q, q):
                nc.gpsimd.collective_compute(
                    "AllGather", mybir.AluOpType.bypass,
                    replica_groups=[ALL],
                    ins=[agin_q[q][:].opt()],
                    outs=[agbuf_q[q][:].opt()],
                )

            # ================= D1: r_i = relu(X @ Wu_i + bu_i)
            with (
                tc.tile_pool(name="d1", bufs=2) as dp,
                tc.tile_pool(name="d1ps", bufs=2, space="PSUM") as pp,
            ):
                for g in range(NG):
                    xt = dp.tile([128, KF, MG * 128], dt.bfloat16, name="xt", tag="xt", bufs=2)
                    for kc in range(KF):
                        nc.sync.dma_start(
                            xt[:, kc, :],
                            featsT[kc * 128:(kc + 1) * 128, g * MG * 128:(g + 1) * MG * 128])
                    for ml in range(MG):
                        m = g * MG + ml
                        ps = pp.tile([128, 3, HP], dt.float32, name="d1p", tag="d1p", bufs=2)
                        for i in range(3):
                            for kc in range(KF):
                                nc.tensor.matmul(
                                    ps[:, i, :],
                                    xt[:, kc, ml * 128:(ml + 1) * 128],
                                    wu_t[:, i, kc, :],
                                    start=(kc == 0), stop=(kc == KF - 1))
                        rt = dp.tile([128, 3, HP], dt.float32, name="rt", tag="rt", bufs=3)
                        r0 = dp.tile([128, HP], dt.bfloat16, name="r0", tag="r0", bufs=3)
                        rf = dp.tile([128, 2, HP], f8, name="rf", tag="rf", bufs=3)
                        for i in range(3):
                            nc.vector.tensor_add(rt[:, i, :], ps[:, i, :], bu_t[:, i, :])
                        nc.vector.tensor_relu(r0[:], rt[:, 0, :])
                        nc.vector.tensor_relu(rf[:, 0, :], rt[:, 1, :])
                        nc.vector.tensor_relu(rf[:, 1, :], rt[:, 2, :])
                        q, lr = divmod(m * 128, QROWS)
                        nc.sync.dma_start(abs1[m * 128:(m + 1) * 128, 0:HP], r0[:])
                        nc.sync.dma_start(agin1[q][lr:lr + 128, 0:HP], rf[:, 0, :])
                        nc.sync.dma_start(agin1[q][lr:lr + 128, HP:W2], rf[:, 1, :])
                        if (m + 1) % BPQ == 0:
                            allgather_q(agin1, agbuf1, q)

            # ================= hop phases
            def hop(agbuf_q, width, emits, next_ag=None, dense_cb=None):
                """emits: list of (col0, dst_kind, dst, dcol)
                   dst_kind: 'abs' -> dst[rows, dcol:dcol+HP] (bf16)
                             'ag'  -> dst[q][lr:lr+128, dcol:dcol+HP] (fp8)
                   next_ag: (agin_q, agbuf_q) to issue per-quarter after the
                   producing blocks complete.
                   dense_cb(b): issue interleaved dense-phase work after
                   block b (overlaps PE/DVE under the gather-bound hop)."""
                with (
                    tc.tile_pool(name="hp", bufs=2) as hp,
                    tc.tile_pool(name="hps", bufs=4, space="PSUM") as hpp,
                ):
                    for b in range(NBLK):
                        bw = segs[b]
                        Cb = sum(C for (_, C) in bw)
                        gt = hp.tile([128, Cb, width], f8, name="gt", tag="gt",
                                     bufs=3)
                        co = 0
                        for w in range(NWIN):
                            off, C = bw[w]
                            while C > 0:
                                Cg = min(C, MAXC)
                                nc.gpsimd.dma_gather(
                                    gt[:, co:co + Cg, :],
                                    agbuf_q[w][:],
                                    idx_t[:, off * 8:(off + Cg) * 8],
                                    num_idxs=Cg * 128,
                                    num_idxs_reg=Cg * 128,
                                    elem_size=width,
                                    single_packet=False,
                                    queue_num=w % NQUEUES)
                                co += Cg
                                off += Cg
                                C -= Cg
                        oh = hp.tile([128, Cb, 128], f8, name="oh", tag="oh",
                                     bufs=2)
                        c0 = bw[0][0]
                        nc.vector.tensor_tensor(
                            oh[:],
                            iota_t[:].unsqueeze(1).broadcast_to([128, Cb, 128]),
                            doff_t[:, c0:c0 + Cb].unsqueeze(2).broadcast_to([128, Cb, 128]),
                            mybir.AluOpType.is_equal)
                        nc.vector.tensor_tensor(
                            oh[:], oh[:],
                            val_t[:, c0:c0 + Cb].unsqueeze(2).broadcast_to([128, Cb, 128]),
                            mybir.AluOpType.mult)
                        ps = hpp.tile([128, width], dt.float32, name="hpsum", tag="hpsum",
                                      bufs=4)
                        for c in range(Cb):
                            nc.tensor.matmul(ps[:], oh[:, c, :], gt[:, c, :],
                                             start=(c == 0), stop=(c == Cb - 1))
                        q, lr = divmod(b * 128, QROWS)
                        for (col0, kind, dst, dcol) in emits:
                            if kind == 'abs':
                                ot = hp.tile([128, HP], dt.bfloat16, name="ot", tag="ot", bufs=4)
                                nc.scalar.activation(ot[:], ps[:, col0:col0 + HP],
                                                     mybir.ActivationFunctionType.Copy,
                                                     scale=DESCALE)
                                nc.sync.dma_start(dst[b * 128:(b + 1) * 128, dcol:dcol + HP], ot[:])
                            else:
                                of = hp.tile([128, HP], f8, name="of", tag="of", bufs=4)
                                nc.scalar.activation(of[:], ps[:, col0:col0 + HP],
                                                     mybir.ActivationFunctionType.Copy,
                                                     scale=DESCALE)
                                nc.sync.dma_start(dst[q][lr:lr + 128, dcol:dcol + HP], of[:])
                        if next_ag is not None and (b + 1) % BPQ == 0:
                            allgather_q(next_ag[0], next_ag[1], q)
                        if dense_cb is not None:
                            dense_cb(b)

            # H1: A[r1|r2] -> Ar1 to abs1 cols HP:2HP ; Ar2 to agin2
            hop(agbuf1, W2, [(0, 'abs', abs1, HP), (HP, 'ag', agin2, 0)],
                next_ag=(agin2, agbuf2))

            # ===== H2 with D2 (B_i = abs1 @ Wb_i) interleaved per group.
            # D2 group g depends only on abs1 rows of H2 blocks
            # g*MG..(g+1)*MG-1, so issue it right after those blocks; its
            # PE/ACT/sync work hides under H2's gather-bound wall, and AG3
            # quarters launch during H2 instead of after it.
            with (
                tc.tile_pool(name="d2", bufs=2) as dp2,
                tc.tile_pool(name="d2ps", bufs=2, space="PSUM") as pp2,
            ):
                def d2_group(g):
                    a1t = dp2.tile([128, KA, MG * 128], dt.bfloat16, name="a1t", tag="a1t", bufs=2)
                    for kc in range(KA):
                        nc.sync.dma_start(
                            a1t[:, kc, :],
                            abs1[g * MG * 128:(g + 1) * MG * 128, kc * 128:(kc + 1) * 128],
                            transpose=True)
                    for ml in range(MG):
                        m = g * MG + ml
                        ps = pp2.tile([128, 3, HP], dt.float32, name="d2p", tag="d2p", bufs=2)
                        for i in range(3):
                            for kc in range(KA):
                                nc.tensor.matmul(
                                    ps[:, i, :],
                                    a1t[:, kc, ml * 128:(ml + 1) * 128],
                                    wb_t[:, i, kc, :],
                                    start=(kc == 0), stop=(kc == KA - 1))
                        b0 = dp2.tile([128, HP], dt.bfloat16, name="b0", tag="b0", bufs=3)
                        bff = dp2.tile([128, 2, HP], f8, name="bff", tag="bff", bufs=3)
                        nc.scalar.activation(b0[:], ps[:, 0, :],
                                             mybir.ActivationFunctionType.Copy)
                        nc.scalar.activation(bff[:, 0, :], ps[:, 1, :],
                                             mybir.ActivationFunctionType.Copy)
                        nc.scalar.activation(bff[:, 1, :], ps[:, 2, :],
                                             mybir.ActivationFunctionType.Copy)
                        q, lr = divmod(m * 128, QROWS)
                        nc.sync.dma_start(abs2[m * 128:(m + 1) * 128, 0:HP], b0[:])
                        nc.sync.dma_start(agin3[q][lr:lr + 128, 0:HP], bff[:, 0, :])
                        nc.sync.dma_start(agin3[q][lr:lr + 128, HP:W2], bff[:, 1, :])
                        if (m + 1) % BPQ == 0:
                            allgather_q(agin3, agbuf3, q)

                # H2: A^2 r2 -> abs1 cols 2HP:3HP
                hop(agbuf2, HP, [(0, 'abs', abs1, 2 * HP)],
                    dense_cb=lambda b: d2_group((b + 1) // MG - 1) if (b + 1) % MG == 0 else None)

            # H3: A[B1|B2] -> AB1 to abs2 ; AB2 to agin4
            hop(agbuf3, W2, [(0, 'abs', abs2, HP), (HP, 'ag', agin4, 0)],
                next_ag=(agin4, agbuf4))

            # ===== H4 with D3 (logits + log_softmax) interleaved per group.
            with (
                tc.tile_pool(name="d3", bufs=2) as dp3,
                tc.tile_pool(name="d3ps", bufs=2, space="PSUM") as pp3,
            ):
                def d3_group(g):
                    a2t = dp3.tile([128, KA, MG * 128], dt.bfloat16, name="a2t", tag="a2t", bufs=2)
                    for kc in range(KA):
                        nc.sync.dma_start(
                            a2t[:, kc, :],
                            abs2[g * MG * 128:(g + 1) * MG * 128, kc * 128:(kc + 1) * 128],
                            transpose=True)
                    for ml in range(MG):
                        m = g * MG + ml
                        ps = pp3.tile([128, CL], dt.float32, name="d3p", tag="d3p", bufs=2)
                        for kc in range(KA):
                            nc.tensor.matmul(
                                ps[:],
                                a2t[:, kc, ml * 128:(ml + 1) * 128],
                                fcw_t[:, kc, :],
                                start=(kc == 0), stop=(kc == KA - 1))
                        lg = dp3.tile([128, CL], dt.float32, name="lg", tag="lg", bufs=3)
                        nc.vector.tensor_add(lg[:], ps[:], fcb_t[:])
                        mx = dp3.tile([128, 1], dt.float32, name="mx", tag="mx", bufs=3)
                        nc.vector.tensor_reduce(mx[:], lg[:], mybir.AxisListType.X,
                                                mybir.AluOpType.max, negate=True)
                        ex = dp3.tile([128, CL], dt.float32, name="ex", tag="ex", bufs=3)
                        sm = dp3.tile([128, 1], dt.float32, name="sm", tag="sm", bufs=3)
                        nc.scalar.activation(ex[:], lg[:], mybir.ActivationFunctionType.Exp,
                                             bias=mx[:], accum_out=sm[:])
                        ln = dp3.tile([128, 1], dt.float32, name="ln", tag="ln", bufs=3)
                        nc.scalar.activation(ln[:], sm[:], mybir.ActivationFunctionType.Ln)
                        ot = dp3.tile([128, CL], dt.float32, name="fot", tag="fot", bufs=3)
                        nc.vector.tensor_scalar(ot[:], lg[:], mx[:], ln[:],
                                                mybir.AluOpType.add,
                                                mybir.AluOpType.subtract)
                        nc.sync.dma_start(y_out[m * 128:(m + 1) * 128, :], ot[:])

                # H4: A^2 B2 -> abs2 cols 2HP:3HP
                hop(agbuf4, HP, [(0, 'abs', abs2, 2 * HP)],
                    dense_cb=lambda b: d3_group((b + 1) // MG - 1) if (b + 1) % MG == 0 else None)

    nc.compile()
    return nc


# ------------------------------------------------------------------- driver

def run(cfg, inputs, trace=False, mode="hw", MAXC=16):
    in_maps, segs, TOT = preprocess(cfg, **inputs)
    nc = build_nc(cfg, segs, TOT, MAXC=MAXC)
    outs = np.zeros((cfg.N, cfg.CL), np.float32)
    if mode == "sim":
        from concourse.bass_interp import MultiCoreSim
        sim = MultiCoreSim(nc, num_cores=cfg.NC, trace=False)
        for c, core in enumerate(sim.cores.values()):
            for k, v in in_maps[c].items():
                core.tensor(k)[:] = v
        sim.simulate()
        for c, core in enumerate(sim.cores.values()):
            outs[c * cfg.RPC_RAW:(c + 1) * cfg.RPC_RAW] = \
                np.asarray(core.tensor("y_out"))[:cfg.RPC_RAW]
        return outs, None
    from concourse import bass_utils
    res = bass_utils.run_bass_kernel_spmd(
        nc, in_maps, core_ids=list(range(cfg.NC)), trace=trace)
    for c in range(cfg.NC):
        outs[c * cfg.RPC_RAW:(c + 1) * cfg.RPC_RAW] = \
            res.results[c]["y_out"][:cfg.RPC_RAW]
    return outs, res


def kernel(**inputs):
    inputs = {k: np.asarray(v) for k, v in inputs.items()}
    out, _ = run(FULL, inputs, trace=False)
    return out
